# revision 2
# baseline (speedup 1.0000x reference)
"""Trainium2 Bass kernel for nn_BilinearSentenceEncoder (v2).

Computes, for sentence [L=128, B=4096, D=300], size [B], W [D, D]:
  sym-pair scores s_{l+1}^T Wsym s_l (Wsym = (W+W.T)/2), self scores
  s_l^T Wsym s_l, 3-way masked softmax over (prev, self, next) channels,
  and the weighted combination out[l] = w1*s[l] + w0*s[l-1] + w2*s[l+1].

Sharding: data-parallel over B across 8 NeuronCores (512 batch columns
per core); W replicated.  Per-core layout: partition dim = L (=128),
free dims = (b, d).

v2 changes vs v1 (1.356 ms):
  - transposes moved OFF the PE onto the DMA XBAR (dma_start_transpose,
    bf16, [128, 16*384] -> st [128, 48, 128] per chunk)
  - V matmuls in bf16 (3 K-chunks per column, N=304) instead of
    fp8 DoubleRow + fp8 cast; back-to-back independent matmuls pipeline
    at 2.4 GHz (~129 ns each) so the fp8 modes buy nothing
  - no fp8/stp/scalar-copy stage at all
"""

import sys

sys.path.insert(0, "/opt/trn_rl_repo")

import numpy as np
import ml_dtypes

import concourse.bacc as bacc
import concourse.mybir as mybir
from concourse import tile
from concourse.bass_utils import run_bass_kernel_spmd

dt = mybir.dt
AF = mybir.ActivationFunctionType
ALU = mybir.AluOpType

L, B, D = 128, 4096, 300
NCORES = 8
BC = B // NCORES          # 512 batch columns per core
CHUNK = 16                # batch columns per pipeline chunk
NCHUNK = BC // CHUNK      # 32
NEG = np.float32(-1.0e38)
DP = 384                  # d padded to %128 for the XBAR transpose
DCH = [(0, 128), (128, 128), (256, 44)]


def _build_nc(profile=False):
    nc = bacc.Bacc()
    f32, bf16 = dt.float32, dt.bfloat16

    s_in = nc.declare_dram_parameter("s", [L, BC, D], f32, isOutput=False)
    m0_in = nc.declare_dram_parameter("m0", [L, BC], f32, isOutput=False)
    m2_in = nc.declare_dram_parameter("m2", [L, BC], f32, isOutput=False)
    wb_in = nc.declare_dram_parameter("wb", [L, 3, 304], bf16, isOutput=False)
    im_in = nc.declare_dram_parameter("imask", [128, 384], dt.uint8, isOutput=False)
    o_out = nc.declare_dram_parameter("o", [L, BC, D], f32, isOutput=True)

    with tile.TileContext(nc) as tc:
        with (
            tc.tile_pool(name="const", bufs=1) as cpool,
            tc.tile_pool(name="s", bufs=3) as s_pool,
            tc.tile_pool(name="sdn", bufs=3) as sdn_pool,
            tc.tile_pool(name="st", bufs=2) as st_pool,
            tc.tile_pool(name="scr", bufs=1) as scr_pool,
            tc.tile_pool(name="sc", bufs=3) as sc_pool,
            tc.tile_pool(name="atb", bufs=2) as atb_pool,
            tc.tile_pool(name="o", bufs=2) as o_pool,
            tc.tile_pool(name="vp", bufs=4, space="PSUM") as v_pool,
            tc.tile_pool(name="op", bufs=2, space="PSUM") as ops_pool,
        ):
            wb_t = cpool.tile([L, 3, 304], bf16)
            im_t = cpool.tile([128, 384], dt.uint8)
            m0_t = cpool.tile([L, BC], f32)
            m2_t = cpool.tile([L, BC], f32)
            nc.sync.dma_start(out=wb_t[:, :, :], in_=wb_in[:, :, :])
            nc.sync.dma_start(out=im_t[:, :], in_=im_in[:, :])
            nc.sync.dma_start(out=m0_t[:, :], in_=m0_in[:, :])
            nc.sync.dma_start(out=m2_t[:, :], in_=m2_in[:, :])

            scr = scr_pool.tile([L, D], f32, tag="scr_a")
            scr2 = scr_pool.tile([L, D], f32, tag="scr_b")

            prev1 = None
            prev2 = None

            def emit_atb(pstate):
                pb0, ps_t, pw1c, pw0up, pw2dn, pc = pstate
                atb = atb_pool.tile([128, CHUNK, 128], bf16)
                with tc.high_priority():
                    if pc < 2:
                        nc.vector.memset(atb[:, :, :], 0.0)
                    im0b = im_t[:, 0:128].unsqueeze(1).broadcast_to([128, CHUNK, 128])
                    imub = im_t[:, 128:256].unsqueeze(1).broadcast_to([128, CHUNK, 128])
                    imdb = im_t[:, 256:384].unsqueeze(1).broadcast_to([128, CHUNK, 128])
                    w1b = pw1c[:, :].unsqueeze(2).broadcast_to([128, CHUNK, 128])
                    w0b = pw0up[:, :].unsqueeze(2).broadcast_to([128, CHUNK, 128])
                    w2b = pw2dn[:, :].unsqueeze(2).broadcast_to([128, CHUNK, 128])
                    nc.vector.copy_predicated(out=atb[:, :, :], mask=im0b, data=w1b)
                    nc.vector.copy_predicated(out=atb[:, :, :], mask=imub, data=w0b)
                    nc.vector.copy_predicated(out=atb[:, :, :], mask=imdb, data=w2b)
                return atb

            def emit_combine(pstate, atb):
                pb0, ps_t, pw1c, pw0up, pw2dn, pc = pstate
                o_t = o_pool.tile([L, CHUNK, D], f32)
                for j in range(CHUNK):
                    ops = ops_pool.tile([128, D], f32)
                    nc.tensor.matmul(
                        ops[:, :],
                        atb[:, j, :],
                        ps_t[:, j, 0:D],
                        start=True,
                        stop=True,
                    )
                    nc.scalar.activation(o_t[:, j, :], ops[:, :], AF.Copy)
                nc.sync.dma_start(out=o_out[:, pb0 : pb0 + CHUNK, :], in_=o_t[:, :, :])

            for c in range(NCHUNK):
                b0 = c * CHUNK
                s_t = s_pool.tile([L, CHUNK, DP], bf16)
                # SWDGE cast-DMA: HBM f32 -> SBUF bf16 (d 300:384 stays stale)
                nc.gpsimd.dma_start(out=s_t[:, :, 0:D], in_=s_in[:, b0 : b0 + CHUNK, :])
                # partition-shifted copy: s_dn[l] = s[l-1]  (row 0 garbage, masked)
                s_dn = sdn_pool.tile([L, CHUNK, D], bf16)
                nc.sync.dma_start(out=s_dn[0:1, :, :], in_=s_t[0:1, :, 0:D])
                for p0 in range(0, 128, 16):
                    pd0, pd1 = max(p0, 1), p0 + 16
                    nc.sync.dma_start(
                        out=s_dn[pd0:pd1, :, :], in_=s_t[pd0 - 1 : pd1 - 1, :, 0:D]
                    )

                # XBAR transpose: st[p, (j,cc), l] = s_t[l, j, 128*cc + p]
                st = st_pool.tile([128, CHUNK, 3, 128], bf16)
                nc.sync.dma_start_transpose(st[:, :, :, :], s_t[:, :, :])

                a_t = sc_pool.tile([L, CHUNK], f32, tag="a_t")
                symdn = sc_pool.tile([L, CHUNK], f32, tag="symdn")

                for j in range(CHUNK):
                    v = v_pool.tile([128, 304], f32)
                    for i, (d0, dn) in enumerate(DCH):
                        nc.tensor.matmul(
                            v[:, :],
                            st[0:dn, j, i, :],
                            wb_t[0:dn, i, :],
                            start=(i == 0),
                            stop=(i == 2),
                        )

                    nc.vector.scalar_tensor_tensor(
                        out=scr[:, :],
                        in0=v[:, 0:D],
                        scalar=1.0 / D,
                        in1=s_t[:, j, 0:D],
                        op0=ALU.mult,
                        op1=ALU.mult,
                        accum_out=a_t[:, j : j + 1],
                    )
                    # symdn[l] = sym[l-1] = <V[l], s[l-1]>/D  (row 0 masked)
                    nc.vector.scalar_tensor_tensor(
                        out=scr2[:, :],
                        in0=v[:, 0:D],
                        scalar=1.0 / D,
                        in1=s_dn[:, j, :],
                        op0=ALU.mult,
                        op1=ALU.mult,
                        accum_out=symdn[:, j : j + 1],
                    )
                    if j == 7 and prev2 is not None:
                        atb_prev = emit_atb(prev2)

                # ---- chunk softmax (batched over CHUNK columns) ----
                sl = slice(b0, b0 + CHUNK)
                sym_t = sc_pool.tile([L, CHUNK], f32, tag="sym_t")
                nc.vector.memset(sym_t[:, :], 0.0)
                nc.sync.dma_start(out=sym_t[0:127, :], in_=symdn[1:128, :])

                l0_t = sc_pool.tile([L, CHUNK], f32, tag="l0")
                l2_t = sc_pool.tile([L, CHUNK], f32, tag="l2")
                nc.vector.tensor_tensor(
                    out=l0_t[:, :], in0=symdn[:, :], in1=m0_t[:, sl], op=ALU.add
                )
                nc.vector.tensor_tensor(
                    out=l2_t[:, :], in0=sym_t[:, :], in1=m2_t[:, sl], op=ALU.add
                )
                e0_t = sc_pool.tile([L, CHUNK], f32, tag="e0")
                e1_t = sc_pool.tile([L, CHUNK], f32, tag="e1")
                e2_t = sc_pool.tile([L, CHUNK], f32, tag="e2")
                nc.scalar.activation(e0_t[:, :], l0_t[:, :], AF.Exp)
                nc.scalar.activation(e1_t[:, :], a_t[:, :], AF.Exp)
                nc.scalar.activation(e2_t[:, :], l2_t[:, :], AF.Exp)
                den_t = sc_pool.tile([L, CHUNK], f32, tag="den")
                nc.vector.tensor_tensor(
                    out=den_t[:, :], in0=e0_t[:, :], in1=e1_t[:, :], op=ALU.add
                )
                nc.vector.tensor_tensor(
                    out=den_t[:, :], in0=den_t[:, :], in1=e2_t[:, :], op=ALU.add
                )
                r_t = sc_pool.tile([L, CHUNK], f32, tag="r")
                nc.vector.reciprocal(r_t[:, :], den_t[:, :])
                w1c = sc_pool.tile([L, CHUNK], f32, tag=f"w1c{c % 3}")
                w0c = sc_pool.tile([L, CHUNK], f32, tag="w0c")
                w2c = sc_pool.tile([L, CHUNK], f32, tag="w2c")
                nc.vector.tensor_tensor(
                    out=w1c[:, :], in0=e1_t[:, :], in1=r_t[:, :], op=ALU.mult
                )
                nc.vector.tensor_tensor(
                    out=w0c[:, :], in0=e0_t[:, :], in1=r_t[:, :], op=ALU.mult
                )
                nc.vector.tensor_tensor(
                    out=w2c[:, :], in0=e2_t[:, :], in1=r_t[:, :], op=ALU.mult
                )
                w0up = sc_pool.tile([L, CHUNK], f32, tag=f"w0up{c % 3}")
                w2dn = sc_pool.tile([L, CHUNK], f32, tag=f"w2dn{c % 3}")
                nc.vector.memset(w0up[:, :], 0.0)
                nc.vector.memset(w2dn[:, :], 0.0)
                nc.sync.dma_start(out=w0up[0:127, :], in_=w0c[1:128, :])
                nc.sync.dma_start(out=w2dn[1:128, :], in_=w2c[0:127, :])

                # ---- deferred combine, two chunks back ----
                if prev2 is not None:
                    emit_combine(prev2, atb_prev)
                prev2 = prev1
                prev1 = (b0, s_t, w1c, w0up, w2dn, c)

            emit_combine(prev2, emit_atb(prev2))
            emit_combine(prev1, emit_atb(prev1))

    nc.compile()
    return nc


_NC_CACHE = {}


def _get_nc():
    if "nc" not in _NC_CACHE:
        _NC_CACHE["nc"] = _build_nc()
    return _NC_CACHE["nc"]


def _host_inputs(sentence, size, W):
    sentence = np.ascontiguousarray(np.asarray(sentence, dtype=np.float32))
    size = np.asarray(size).astype(np.int64)
    W = np.asarray(W, dtype=np.float32)

    wsym = 0.5 * (W + W.T)
    # wb[p, i, :] = wsym[128*i + p, :] zero-padded to 304 cols
    wb = np.zeros((128, 3, 304), dtype=ml_dtypes.bfloat16)
    for i, (d0, dn) in enumerate(DCH):
        wb[0:dn, i, 0:D] = wsym[d0 : d0 + dn, :].astype(ml_dtypes.bfloat16)

    I0 = np.eye(128, dtype=np.float32)
    Iup = np.zeros((128, 128), np.float32)
    Iup[np.arange(127), np.arange(1, 128)] = 1.0
    Idn = np.zeros((128, 128), np.float32)
    Idn[np.arange(1, 128), np.arange(127)] = 1.0
    imask = np.ascontiguousarray(
        np.concatenate([I0, Iup, Idn], axis=1).astype(np.uint8)
    )

    pos = np.arange(L, dtype=np.int64)[:, None]
    m0 = np.where(pos < size[None, :], 0.0, NEG).astype(np.float32)
    m0[0, :] = NEG
    m2 = np.where(pos < np.clip(size - 1, 0, None)[None, :], 0.0, NEG).astype(
        np.float32
    )
    m2[L - 1, :] = NEG

    in_maps = []
    for c in range(NCORES):
        bsl = slice(c * BC, (c + 1) * BC)
        in_maps.append(
            {
                "s": np.ascontiguousarray(sentence[:, bsl, :]),
                "m0": np.ascontiguousarray(m0[:, bsl]),
                "m2": np.ascontiguousarray(m2[:, bsl]),
                "wb": wb,
                "imask": imask,
            }
        )
    return in_maps


def kernel(sentence, size, W):
    nc = _get_nc()
    in_maps = _host_inputs(sentence, size, W)
    res = run_bass_kernel_spmd(nc, in_maps, core_ids=list(range(NCORES)))
    out = np.concatenate([res.results[c]["o"] for c in range(NCORES)], axis=1)
    return out.astype(np.float32)


def _install_ntff_hook():
    """Register the axon NTFF profiling hook that this container's boot
    skipped (antenv.axon_hooks module absent)."""
    try:
        from antenv.axon_hooks import get_axon_ntff_profile_hook  # noqa: F401

        return
    except ImportError:
        pass
    import contextlib
    import ctypes
    import types

    so_path = "/opt/axon/libaxon_pjrt.so"
    lib = ctypes.CDLL(so_path)
    if not hasattr(lib, "axon_start_nrt_profile"):
        return
    lib.axon_start_nrt_profile.argtypes = [
        ctypes.POINTER(ctypes.c_int64),
        ctypes.c_size_t,
    ]
    lib.axon_start_nrt_profile.restype = ctypes.c_int64
    lib.axon_stop_nrt_profile.argtypes = [ctypes.c_char_p]
    lib.axon_stop_nrt_profile.restype = ctypes.c_int64

    @contextlib.contextmanager
    def _hook(output_dir, device_ids):
        import jax

        jax.devices()
        if device_ids:
            ids = (ctypes.c_int64 * len(device_ids))(*device_ids)
            rc = lib.axon_start_nrt_profile(ids, len(device_ids))
        else:
            rc = lib.axon_start_nrt_profile(None, 0)
        if rc != 0:
            raise RuntimeError(f"axon_start_nrt_profile rc={rc}")
        try:
            yield
        finally:
            n = lib.axon_stop_nrt_profile(str(output_dir).encode())
            print(f"ntff capture: {n} file(s) -> {output_dir}")

    mod = types.ModuleType("antenv.axon_hooks")
    mod.get_axon_ntff_profile_hook = lambda: _hook
    mod.set_axon_ntff_profile_hook = lambda h: None
    import antenv

    sys.modules["antenv.axon_hooks"] = mod
    antenv.axon_hooks = mod


def run_traced(sentence, size, W):
    """Like kernel(), but also returns (exec_time_ns, profile_json path)."""
    _install_ntff_hook()
    nc = _get_nc()
    in_maps = _host_inputs(sentence, size, W)
    res = run_bass_kernel_spmd(
        nc, in_maps, core_ids=list(range(NCORES)), trace=True, trace_cores=[0]
    )
    out = np.concatenate([res.results[c]["o"] for c in range(NCORES)], axis=1)
    return out.astype(np.float32), res.exec_time_ns, res.profile_json


if __name__ == "__main__":
    rng = np.random.default_rng(0)
    s = rng.standard_normal((L, B, D)).astype(np.float32)
    sz = rng.integers(0, L, size=(B,)).astype(np.int32)
    W = (rng.standard_normal((D, D)) / np.sqrt(D)).astype(np.float32)
    out = kernel(s, sz, W)
    print("out", out.shape, out.dtype, np.abs(out).max())


# revision 4
# speedup vs baseline: 1.2594x; 1.2594x over previous
"""Trainium2 Bass kernel for nn_BilinearSentenceEncoder (v2).

Computes, for sentence [L=128, B=4096, D=300], size [B], W [D, D]:
  sym-pair scores s_{l+1}^T Wsym s_l (Wsym = (W+W.T)/2), self scores
  s_l^T Wsym s_l, 3-way masked softmax over (prev, self, next) channels,
  and the weighted combination out[l] = w1*s[l] + w0*s[l-1] + w2*s[l+1].

Sharding: data-parallel over B across 8 NeuronCores (512 batch columns
per core); W replicated.  Per-core layout: partition dim = L (=128),
free dims = (b, d).

v2 changes vs v1 (1.356 ms):
  - transposes moved OFF the PE onto the DMA XBAR (dma_start_transpose,
    bf16, [128, 16*384] -> st [128, 48, 128] per chunk)
  - V matmuls in bf16 (3 K-chunks per column, N=304) instead of
    fp8 DoubleRow + fp8 cast; back-to-back independent matmuls pipeline
    at 2.4 GHz (~129 ns each) so the fp8 modes buy nothing
  - no fp8/stp/scalar-copy stage at all
"""

import sys

sys.path.insert(0, "/opt/trn_rl_repo")

import numpy as np
import ml_dtypes

import concourse.bacc as bacc
import concourse.mybir as mybir
from concourse import tile
from concourse.bass_utils import run_bass_kernel_spmd

dt = mybir.dt
AF = mybir.ActivationFunctionType
ALU = mybir.AluOpType

L, B, D = 128, 4096, 300
NCORES = 8
BC = B // NCORES          # 512 batch columns per core
CHUNK = 16                # batch columns per pipeline chunk
NCHUNK = BC // CHUNK      # 32
NEG = np.float32(-1.0e38)
DP = 384                  # d padded to %128 for the XBAR transpose
DCH = [(0, 128), (128, 128), (256, 44)]


def _build_nc(profile=False):
    nc = bacc.Bacc()
    f32, bf16 = dt.float32, dt.bfloat16

    s_in = nc.declare_dram_parameter("s", [L, BC, D], f32, isOutput=False)
    m0_in = nc.declare_dram_parameter("m0", [L, BC], f32, isOutput=False)
    m2_in = nc.declare_dram_parameter("m2", [L, BC], f32, isOutput=False)
    wb_in = nc.declare_dram_parameter("wb", [L, 3, 304], bf16, isOutput=False)
    im_in = nc.declare_dram_parameter("imask", [128, 384], dt.uint8, isOutput=False)
    o_out = nc.declare_dram_parameter("o", [L, BC, D], f32, isOutput=True)

    with tile.TileContext(nc) as tc:
        with (
            tc.tile_pool(name="const", bufs=1) as cpool,
            tc.tile_pool(name="s", bufs=3) as s_pool,
            tc.tile_pool(name="sdn", bufs=3) as sdn_pool,
            tc.tile_pool(name="st", bufs=2) as st_pool,
            tc.tile_pool(name="scr", bufs=1) as scr_pool,
            tc.tile_pool(name="sc", bufs=3) as sc_pool,
            tc.tile_pool(name="atb", bufs=2) as atb_pool,
            tc.tile_pool(name="o", bufs=2) as o_pool,
            tc.tile_pool(name="vp", bufs=4, space="PSUM") as v_pool,
            tc.tile_pool(name="op", bufs=2, space="PSUM") as ops_pool,
        ):
            wb_t = cpool.tile([L, 3, 304], bf16)
            im_t = cpool.tile([128, 384], dt.uint8)
            m0_t = cpool.tile([L, BC], f32)
            m2_t = cpool.tile([L, BC], f32)
            nc.sync.dma_start(out=wb_t[:, :, :], in_=wb_in[:, :, :])
            nc.sync.dma_start(out=im_t[:, :], in_=im_in[:, :])
            nc.sync.dma_start(out=m0_t[:, :], in_=m0_in[:, :])
            nc.sync.dma_start(out=m2_t[:, :], in_=m2_in[:, :])

            scr = scr_pool.tile([L, D], f32, tag="scr_a")
            scr2 = scr_pool.tile([L, D], f32, tag="scr_b")

            prev1 = None
            prev2 = None

            def emit_atb(pstate):
                pb0, ps_t, pw1c, pw0up, pw2dn, pc = pstate
                atb = atb_pool.tile([128, CHUNK, 128], bf16)
                with tc.high_priority():
                    if pc < 2:
                        nc.vector.memset(atb[:, :, :], 0.0)
                    im0b = im_t[:, 0:128].unsqueeze(1).broadcast_to([128, CHUNK, 128])
                    imub = im_t[:, 128:256].unsqueeze(1).broadcast_to([128, CHUNK, 128])
                    imdb = im_t[:, 256:384].unsqueeze(1).broadcast_to([128, CHUNK, 128])
                    w1b = pw1c[:, :].unsqueeze(2).broadcast_to([128, CHUNK, 128])
                    w0b = pw0up[:, :].unsqueeze(2).broadcast_to([128, CHUNK, 128])
                    w2b = pw2dn[:, :].unsqueeze(2).broadcast_to([128, CHUNK, 128])
                    nc.vector.copy_predicated(out=atb[:, :, :], mask=im0b, data=w1b)
                    nc.vector.copy_predicated(out=atb[:, :, :], mask=imub, data=w0b)
                    nc.vector.copy_predicated(out=atb[:, :, :], mask=imdb, data=w2b)
                return atb

            def emit_combine(pstate, atb):
                pb0, ps_t, pw1c, pw0up, pw2dn, pc = pstate
                o_t = o_pool.tile([L, CHUNK, D], f32)
                for j in range(CHUNK):
                    ops = ops_pool.tile([128, D], f32)
                    nc.tensor.matmul(
                        ops[:, :],
                        atb[:, j, :],
                        ps_t[:, j, 0:D],
                        start=True,
                        stop=True,
                    )
                    nc.scalar.activation(o_t[:, j, :], ops[:, :], AF.Copy)
                nc.sync.dma_start(out=o_out[:, pb0 : pb0 + CHUNK, :], in_=o_t[:, :, :])

            for c in range(NCHUNK):
                b0 = c * CHUNK
                s_t = s_pool.tile([L, CHUNK, DP], bf16)
                # SWDGE cast-DMA: HBM f32 -> SBUF bf16 (d 300:384 stays stale)
                nc.gpsimd.dma_start(out=s_t[:, :, 0:D], in_=s_in[:, b0 : b0 + CHUNK, :])
                # partition-shifted copy: s_dn[l] = s[l-1]  (row 0 garbage, masked)
                s_dn = sdn_pool.tile([L, CHUNK, D], bf16)
                nc.sync.dma_start(out=s_dn[0:1, :, :], in_=s_t[0:1, :, 0:D])
                for qi, p0 in enumerate(range(0, 128, 16)):
                    pd0, pd1 = max(p0, 1), p0 + 16
                    eng = nc.sync if qi % 2 == 0 else nc.scalar
                    eng.dma_start(
                        out=s_dn[pd0:pd1, :, :], in_=s_t[pd0 - 1 : pd1 - 1, :, 0:D]
                    )

                # XBAR transpose: st[p, (j,cc), l] = s_t[l, j, 128*cc + p]
                st = st_pool.tile([128, CHUNK, 3, 128], bf16)
                nc.scalar.dma_start_transpose(st[:, :, :, :], s_t[:, :, :])

                a_t = sc_pool.tile([L, CHUNK], f32, tag="a_t")
                symdn = sc_pool.tile([L, CHUNK], f32, tag="symdn")

                for j in range(CHUNK):
                    v = v_pool.tile([128, 304], f32)
                    for i, (d0, dn) in enumerate(DCH):
                        nc.tensor.matmul(
                            v[:, :],
                            st[0:dn, j, i, :],
                            wb_t[0:dn, i, :],
                            start=(i == 0),
                            stop=(i == 2),
                        )

                    nc.vector.scalar_tensor_tensor(
                        out=scr[:, :],
                        in0=v[:, 0:D],
                        scalar=1.0 / D,
                        in1=s_t[:, j, 0:D],
                        op0=ALU.mult,
                        op1=ALU.mult,
                        accum_out=a_t[:, j : j + 1],
                    )
                    # symdn[l] = sym[l-1] = <V[l], s[l-1]>/D  (row 0 masked)
                    nc.vector.scalar_tensor_tensor(
                        out=scr2[:, :],
                        in0=v[:, 0:D],
                        scalar=1.0 / D,
                        in1=s_dn[:, j, :],
                        op0=ALU.mult,
                        op1=ALU.mult,
                        accum_out=symdn[:, j : j + 1],
                    )
                    if j == 7 and prev2 is not None:
                        atb_prev = emit_atb(prev2)

                # ---- chunk softmax (batched over CHUNK columns) ----
                sl = slice(b0, b0 + CHUNK)
                sym_t = sc_pool.tile([L, CHUNK], f32, tag="sym_t")
                nc.vector.memset(sym_t[:, :], 0.0)
                nc.sync.dma_start(out=sym_t[0:127, :], in_=symdn[1:128, :])

                l0_t = sc_pool.tile([L, CHUNK], f32, tag="l0")
                l2_t = sc_pool.tile([L, CHUNK], f32, tag="l2")
                nc.vector.tensor_tensor(
                    out=l0_t[:, :], in0=symdn[:, :], in1=m0_t[:, sl], op=ALU.add
                )
                nc.vector.tensor_tensor(
                    out=l2_t[:, :], in0=sym_t[:, :], in1=m2_t[:, sl], op=ALU.add
                )
                e0_t = sc_pool.tile([L, CHUNK], f32, tag="e0")
                e1_t = sc_pool.tile([L, CHUNK], f32, tag="e1")
                e2_t = sc_pool.tile([L, CHUNK], f32, tag="e2")
                nc.scalar.activation(e0_t[:, :], l0_t[:, :], AF.Exp)
                nc.scalar.activation(e1_t[:, :], a_t[:, :], AF.Exp)
                nc.scalar.activation(e2_t[:, :], l2_t[:, :], AF.Exp)
                den_t = sc_pool.tile([L, CHUNK], f32, tag="den")
                nc.vector.tensor_tensor(
                    out=den_t[:, :], in0=e0_t[:, :], in1=e1_t[:, :], op=ALU.add
                )
                nc.vector.tensor_tensor(
                    out=den_t[:, :], in0=den_t[:, :], in1=e2_t[:, :], op=ALU.add
                )
                r_t = sc_pool.tile([L, CHUNK], f32, tag="r")
                nc.vector.reciprocal(r_t[:, :], den_t[:, :])
                w1c = sc_pool.tile([L, CHUNK], f32, tag=f"w1c{c % 3}")
                w0c = sc_pool.tile([L, CHUNK], f32, tag="w0c")
                w2c = sc_pool.tile([L, CHUNK], f32, tag="w2c")
                nc.vector.tensor_tensor(
                    out=w1c[:, :], in0=e1_t[:, :], in1=r_t[:, :], op=ALU.mult
                )
                nc.vector.tensor_tensor(
                    out=w0c[:, :], in0=e0_t[:, :], in1=r_t[:, :], op=ALU.mult
                )
                nc.vector.tensor_tensor(
                    out=w2c[:, :], in0=e2_t[:, :], in1=r_t[:, :], op=ALU.mult
                )
                w0up = sc_pool.tile([L, CHUNK], f32, tag=f"w0up{c % 3}")
                w2dn = sc_pool.tile([L, CHUNK], f32, tag=f"w2dn{c % 3}")
                nc.vector.memset(w0up[:, :], 0.0)
                nc.vector.memset(w2dn[:, :], 0.0)
                nc.sync.dma_start(out=w0up[0:127, :], in_=w0c[1:128, :])
                nc.sync.dma_start(out=w2dn[1:128, :], in_=w2c[0:127, :])

                # ---- deferred combine, two chunks back ----
                if prev2 is not None:
                    emit_combine(prev2, atb_prev)
                prev2 = prev1
                prev1 = (b0, s_t, w1c, w0up, w2dn, c)

            emit_combine(prev2, emit_atb(prev2))
            emit_combine(prev1, emit_atb(prev1))

    nc.compile()
    return nc


_NC_CACHE = {}


def _get_nc():
    if "nc" not in _NC_CACHE:
        _NC_CACHE["nc"] = _build_nc()
    return _NC_CACHE["nc"]


def _host_inputs(sentence, size, W):
    sentence = np.ascontiguousarray(np.asarray(sentence, dtype=np.float32))
    size = np.asarray(size).astype(np.int64)
    W = np.asarray(W, dtype=np.float32)

    wsym = 0.5 * (W + W.T)
    # wb[p, i, :] = wsym[128*i + p, :] zero-padded to 304 cols
    wb = np.zeros((128, 3, 304), dtype=ml_dtypes.bfloat16)
    for i, (d0, dn) in enumerate(DCH):
        wb[0:dn, i, 0:D] = wsym[d0 : d0 + dn, :].astype(ml_dtypes.bfloat16)

    I0 = np.eye(128, dtype=np.float32)
    Iup = np.zeros((128, 128), np.float32)
    Iup[np.arange(127), np.arange(1, 128)] = 1.0
    Idn = np.zeros((128, 128), np.float32)
    Idn[np.arange(1, 128), np.arange(127)] = 1.0
    imask = np.ascontiguousarray(
        np.concatenate([I0, Iup, Idn], axis=1).astype(np.uint8)
    )

    pos = np.arange(L, dtype=np.int64)[:, None]
    m0 = np.where(pos < size[None, :], 0.0, NEG).astype(np.float32)
    m0[0, :] = NEG
    m2 = np.where(pos < np.clip(size - 1, 0, None)[None, :], 0.0, NEG).astype(
        np.float32
    )
    m2[L - 1, :] = NEG

    in_maps = []
    for c in range(NCORES):
        bsl = slice(c * BC, (c + 1) * BC)
        in_maps.append(
            {
                "s": np.ascontiguousarray(sentence[:, bsl, :]),
                "m0": np.ascontiguousarray(m0[:, bsl]),
                "m2": np.ascontiguousarray(m2[:, bsl]),
                "wb": wb,
                "imask": imask,
            }
        )
    return in_maps


def kernel(sentence, size, W):
    nc = _get_nc()
    in_maps = _host_inputs(sentence, size, W)
    res = run_bass_kernel_spmd(nc, in_maps, core_ids=list(range(NCORES)))
    out = np.concatenate([res.results[c]["o"] for c in range(NCORES)], axis=1)
    return out.astype(np.float32)


def _install_ntff_hook():
    """Register the axon NTFF profiling hook that this container's boot
    skipped (antenv.axon_hooks module absent)."""
    try:
        from antenv.axon_hooks import get_axon_ntff_profile_hook  # noqa: F401

        return
    except ImportError:
        pass
    import contextlib
    import ctypes
    import types

    so_path = "/opt/axon/libaxon_pjrt.so"
    lib = ctypes.CDLL(so_path)
    if not hasattr(lib, "axon_start_nrt_profile"):
        return
    lib.axon_start_nrt_profile.argtypes = [
        ctypes.POINTER(ctypes.c_int64),
        ctypes.c_size_t,
    ]
    lib.axon_start_nrt_profile.restype = ctypes.c_int64
    lib.axon_stop_nrt_profile.argtypes = [ctypes.c_char_p]
    lib.axon_stop_nrt_profile.restype = ctypes.c_int64

    @contextlib.contextmanager
    def _hook(output_dir, device_ids):
        import jax

        jax.devices()
        if device_ids:
            ids = (ctypes.c_int64 * len(device_ids))(*device_ids)
            rc = lib.axon_start_nrt_profile(ids, len(device_ids))
        else:
            rc = lib.axon_start_nrt_profile(None, 0)
        if rc != 0:
            raise RuntimeError(f"axon_start_nrt_profile rc={rc}")
        try:
            yield
        finally:
            n = lib.axon_stop_nrt_profile(str(output_dir).encode())
            print(f"ntff capture: {n} file(s) -> {output_dir}")

    mod = types.ModuleType("antenv.axon_hooks")
    mod.get_axon_ntff_profile_hook = lambda: _hook
    mod.set_axon_ntff_profile_hook = lambda h: None
    import antenv

    sys.modules["antenv.axon_hooks"] = mod
    antenv.axon_hooks = mod


def run_traced(sentence, size, W):
    """Like kernel(), but also returns (exec_time_ns, profile_json path)."""
    _install_ntff_hook()
    nc = _get_nc()
    in_maps = _host_inputs(sentence, size, W)
    res = run_bass_kernel_spmd(
        nc, in_maps, core_ids=list(range(NCORES)), trace=True, trace_cores=[0]
    )
    out = np.concatenate([res.results[c]["o"] for c in range(NCORES)], axis=1)
    return out.astype(np.float32), res.exec_time_ns, res.profile_json


if __name__ == "__main__":
    rng = np.random.default_rng(0)
    s = rng.standard_normal((L, B, D)).astype(np.float32)
    sz = rng.integers(0, L, size=(B,)).astype(np.int32)
    W = (rng.standard_normal((D, D)) / np.sqrt(D)).astype(np.float32)
    out = kernel(s, sz, W)
    print("out", out.shape, out.dtype, np.abs(out).max())
